# revision 35
# baseline (speedup 1.0000x reference)
"""Trainium2 Bass kernel: causal sliding-window GQA self-attention.

Problem: B=2, T=2048, C=2048, 16 q-heads / 4 kv-heads, head_dim=128,
RoPE, sliding window 512, projections Wq/Wk/Wv/Wo.

Sharding: 8 cores = DP(batch=2) x TP(head-groups=4).  Core c handles
batch c//4 and q-heads [4*(c%4), 4*(c%4)+4) (one kv head c%4).  Each
core computes a partial output contribution [T, C]; the host sums the
4 head-group partials per batch.

Per-core kernel (all matmuls bf16, f32 accumulation):
  - host passes x[b]^T so the contraction dim (C) lands on partitions
  - Q^T/K^T [hd, t] via PE matmul, RoPE applied during PSUM eviction
    (scale 1/sqrt(hd) folded into the Q rope tables; the rotate-half is
    a cross-partition DVE multiply against sign-folded sin tables, and
    the final add runs on GpSimd to keep DVE off the critical path)
  - V^T computed, then PE-transposed to V [t, hd]
  - per (head, 128-query block): scores S^T [j, q] for the <=5 key
    blocks of the 640-wide causal window, one batched exp on ScalarE
    (no max subtraction -- max |score| measured 5.5 on this input
    distribution, exp stays tiny vs f32/bf16 range), 0/1 band-mask
    multiply (only the two edge key-blocks are not all-ones), PV and
    ones-row-sum matmuls as sequential accumulation groups in one PSUM
    bank (start=True clears has_written for the whole 2KB zero region,
    so groups sharing a bank must not interleave), bf16 reciprocal +
    GpSimd partition_broadcast for the 1/s normalization fused into the
    y^T eviction, then per-head Wo matmul accumulated over the 4 heads
    in PSUM.  DMA emission order is tuned so the shared 360GB/s DMA
    path stays ahead of the PE during the projection phase.

Timeline-sim (cost model) per-core exec: ~224.5us; rel err vs the f32
reference ~4.5e-3 (bf16 quantization of inputs/intermediates).
"""

import os
import sys

for _p in ("/opt/trn_rl_repo", "/root/.axon_site/_ro/trn_rl_repo"):
    if os.path.isdir(_p) and _p not in sys.path:
        sys.path.append(_p)

import numpy as np
import ml_dtypes

BF16 = ml_dtypes.bfloat16

B, T, C = 2, 2048, 2048
H, KVH, HD = 16, 4, 128
WIN = 512
ROPE_BASE = 10000.0
NCORES = 8
TPG = 4           # tensor-parallel group count (head groups)
HPG = H // TPG    # q-heads per core
SCALE = 1.0 / float(np.sqrt(np.float32(HD)))
NWINB = WIN // 128 + 1   # 5 key blocks cover the 640-wide window

_NC_CACHE = {}


def _rope_tables(t_len):
    # Match reference: angles computed in float32.
    inv = (1.0 / (np.float32(ROPE_BASE) ** (np.arange(0, HD, 2, dtype=np.float32) / np.float32(HD)))).astype(np.float32)
    ang = np.arange(t_len, dtype=np.float32)[None, :] * inv[:, None]   # [64, T]
    cosT = np.concatenate([np.cos(ang), np.cos(ang)], axis=0)          # [128, T]
    sinT = np.sin(ang)
    sin_swap = np.concatenate([-sinT, sinT], axis=0)                   # [128, T]
    return cosT.astype(np.float32), sin_swap.astype(np.float32)


def _band_maskT():
    # maskT[c, r] = 1 iff query row r may attend key col c of the
    # 640-wide window (c = j - (qs - 512)):  r+1 <= c <= r+512.
    r = np.arange(128)[None, :]
    c = np.arange(NWINB * 128)[:, None]
    return ((r + 1 <= c) & (c <= r + WIN)).astype(np.float32)          # [640, 128]


def build_nc(t_len=T):
    """Build + compile the per-core Bass module (SPMD, identical on all cores)."""
    import concourse.mybir as mybir
    import concourse.tile as tile
    from concourse import bacc

    dt = mybir.dt
    NQB = t_len // 128        # query/key blocks
    NCB = C // 128            # contraction blocks for projections
    NTB = t_len // 512        # 512-wide t-blocks for projections

    nc = bacc.Bacc("TRN2", target_bir_lowering=False, debug=False, num_devices=NCORES)

    def din(name, shape, d=dt.bfloat16):
        return nc.dram_tensor(name, shape, d, kind="ExternalInput").ap()

    xT_d = din("xT", [C, t_len])
    wq_d = din("wq", [C, HPG * HD])
    wk_d = din("wk", [C, HD])
    wv_d = din("wv", [C, HD])
    wo_d = din("wo", [HPG * HD, C])
    cosq_d = din("cosq", [HD, t_len])
    sinq_d = din("sinq", [HD, t_len])
    cosk_d = din("cosk", [HD, t_len])
    sink_d = din("sink", [HD, t_len])
    maskT_d = din("maskT", [NWINB * 128, 128])
    ident_d = din("ident", [128, 128])
    onesj_d = din("ones_j", [128, 1])
    out_d = nc.dram_tensor("out", [t_len, C], dt.float32, kind="ExternalOutput").ap()

    with tile.TileContext(nc) as tc:
        with tc.tile_pool(name="persist", bufs=1) as pp:
            xT_sb = pp.tile([128, NCB * t_len], dt.bfloat16, tag="xT")
            wq_sb = pp.tile([128, NCB * HPG * HD], dt.bfloat16, tag="wq")
            wk_sb = pp.tile([128, NCB * HD], dt.bfloat16, tag="wk")
            wv_sb = pp.tile([128, NCB * HD], dt.bfloat16, tag="wv")
            wo_sb = pp.tile([128, HPG * C], dt.bfloat16, tag="wo")
            QT_sb = [pp.tile([128, t_len], dt.bfloat16, tag=f"QT{h}", name=f"QT{h}") for h in range(HPG)]
            KT_sb = pp.tile([128, t_len], dt.bfloat16, tag="KT")
            VT_sb = pp.tile([128, t_len], dt.bfloat16, tag="VT")
            V_sb = pp.tile([128, t_len], dt.bfloat16, tag="V")
            cosq_sb = pp.tile([128, t_len], dt.bfloat16, tag="cosq")
            sinq_sb = pp.tile([128, t_len], dt.bfloat16, tag="sinq")
            cosk_sb = pp.tile([128, t_len], dt.bfloat16, tag="cosk")
            sink_sb = pp.tile([128, t_len], dt.bfloat16, tag="sink")
            maskT_sb = pp.tile([128, NWINB * 128], dt.bfloat16, tag="maskT")
            ident_sb = pp.tile([128, 128], dt.bfloat16, tag="ident")
            onesj_sb = pp.tile([128, 1], dt.bfloat16, tag="onesj")

            # Load order matters: the shared DMA path is the projection-phase
            # rate limiter.  Small constants + rope tables + V/K weights first
            # (first consumers), then xT/wq interleaved, wo (attention-only) last.
            nc.sync.dma_start(ident_sb[:], ident_d[:])
            nc.sync.dma_start(onesj_sb[:], onesj_d[:])
            for cb in range(NCB):
                nc.sync.dma_start(xT_sb[:, cb * t_len:(cb + 1) * t_len], xT_d[cb * 128:(cb + 1) * 128, :])
                nc.sync.dma_start(wv_sb[:, cb * HD:(cb + 1) * HD], wv_d[cb * 128:(cb + 1) * 128, :])
                nc.sync.dma_start(wk_sb[:, cb * HD:(cb + 1) * HD], wk_d[cb * 128:(cb + 1) * 128, :])
            nc.sync.dma_start(cosk_sb[:], cosk_d[:])
            nc.sync.dma_start(sink_sb[:], sink_d[:])
            for cb in range(NCB):
                nc.sync.dma_start(wq_sb[:, cb * HPG * HD:(cb + 1) * HPG * HD], wq_d[cb * 128:(cb + 1) * 128, :])
                if cb == 3:
                    nc.sync.dma_start(cosq_sb[:], cosq_d[:])
                    nc.sync.dma_start(sinq_sb[:], sinq_d[:])
            for m in range(NWINB):
                nc.sync.dma_start(maskT_sb[:, m * 128:(m + 1) * 128], maskT_d[m * 128:(m + 1) * 128, :])
            for h in range(HPG):
                nc.sync.dma_start(wo_sb[:, h * C:(h + 1) * C], wo_d[h * 128:(h + 1) * 128, :])

            # ---------------- projections ----------------
            # Order: V -> V-transpose -> K -> Q(head-major).  Attention for
            # (h=0, qb=0) unblocks as soon as the first Q eviction lands.
            with tc.tile_pool(name="proj_ps", bufs=6, space="PSUM") as pps, \
                 tc.tile_pool(name="rope_scr", bufs=4) as rsc:

                def rope_evict(ps, dst, cos_sb, sin_sb, tb):
                    sl = slice(tb * 512, (tb + 1) * 512)
                    t1 = rsc.tile([128, 512], dt.float32, tag="t1")
                    t2 = rsc.tile([128, 512], dt.float32, tag="t2")
                    nc.vector.tensor_mul(t1[:], ps[:], cos_sb[:, sl])
                    nc.vector.tensor_mul(t2[0:64, :], ps[64:128, :], sin_sb[0:64, sl])
                    nc.vector.tensor_mul(t2[64:128, :], ps[0:64, :], sin_sb[64:128, sl])
                    nc.gpsimd.tensor_add(dst, t1[:], t2[:])

                for tb in range(NTB):
                    ps = pps.tile([128, 512], dt.float32, tag="ps")
                    for cb in range(NCB):
                        nc.tensor.matmul(
                            ps[:], wv_sb[:, cb * HD:(cb + 1) * HD],
                            xT_sb[:, cb * t_len + tb * 512: cb * t_len + (tb + 1) * 512],
                            start=(cb == 0), stop=(cb == NCB - 1))
                    nc.any.tensor_copy(VT_sb[:, tb * 512:(tb + 1) * 512], ps[:])
                with tc.tile_pool(name="tr_ps", bufs=2, space="PSUM") as tps:
                    for jb in range(NQB):
                        tp = tps.tile([128, 128], dt.bfloat16, tag="tp")
                        nc.tensor.transpose(tp[:], VT_sb[:, jb * 128:(jb + 1) * 128], ident_sb[:])
                        nc.any.tensor_copy(V_sb[:, jb * 128:(jb + 1) * 128], tp[:])
                for tb in range(NTB):
                    ps = pps.tile([128, 512], dt.float32, tag="ps")
                    for cb in range(NCB):
                        nc.tensor.matmul(
                            ps[:], wk_sb[:, cb * HD:(cb + 1) * HD],
                            xT_sb[:, cb * t_len + tb * 512: cb * t_len + (tb + 1) * 512],
                            start=(cb == 0), stop=(cb == NCB - 1))
                    rope_evict(ps, KT_sb[:, tb * 512:(tb + 1) * 512], cosk_sb, sink_sb, tb)
                for h in range(HPG):
                    for tb in range(NTB):
                        ps = pps.tile([128, 512], dt.float32, tag="ps")
                        for cb in range(NCB):
                            nc.tensor.matmul(
                                ps[:],
                                wq_sb[:, cb * HPG * HD + h * HD: cb * HPG * HD + (h + 1) * HD],
                                xT_sb[:, cb * t_len + tb * 512: cb * t_len + (tb + 1) * 512],
                                start=(cb == 0), stop=(cb == NCB - 1))
                        rope_evict(ps, QT_sb[h][:, tb * 512:(tb + 1) * 512],
                                   cosq_sb, sinq_sb, tb)

            # ---------------- attention + Wo ----------------
            with tc.tile_pool(name="st_ps", bufs=2, space="PSUM") as stp, \
                 tc.tile_pool(name="acc_ps", bufs=2, space="PSUM") as accp, \
                 tc.tile_pool(name="wo_ps", bufs=2, space="PSUM") as wop, \
                 tc.tile_pool(name="attn_sb", bufs=12) as asb, \
                 tc.tile_pool(name="yn_sb", bufs=2) as ysb, \
                 tc.tile_pool(name="out_sb", bufs=2) as osb:
                Exp = mybir.ActivationFunctionType.Exp
                for qb in range(NQB):
                    nwin = min(qb, NWINB - 1) + 1
                    ynT = ysb.tile([128, HPG * 128], dt.bfloat16, tag="ynT")
                    for h in range(HPG):
                        qsl = slice(qb * 128, (qb + 1) * 128)
                        st = stp.tile([128, NWINB * 128], dt.float32, tag="st")
                        for i in range(nwin):
                            jb = qb - nwin + 1 + i
                            nc.tensor.matmul(
                                st[:, i * 128:(i + 1) * 128],
                                KT_sb[:, jb * 128:(jb + 1) * 128],
                                QT_sb[h][:, qsl], start=True, stop=True)
                        # acc bank layout: [:,0:128] y^T, [0:1,128:256] s
                        # NB: groups into this bank must be sequential (start=True
                        # clears has_written for the whole 2KB zero region).
                        acc = accp.tile([128, 256], dt.float32, tag="acc")
                        pexp = asb.tile([128, NWINB * 128], dt.bfloat16, tag="pexp")
                        nc.scalar.activation(pexp[:, 0:nwin * 128], st[:, 0:nwin * 128], Exp)
                        pms = []
                        for i in range(nwin):
                            m = i + NWINB - nwin
                            if m == 0 or m == NWINB - 1:
                                pm = asb.tile([128, 128], dt.bfloat16, tag="pmask")
                                nc.vector.tensor_mul(pm[:], pexp[:, i * 128:(i + 1) * 128],
                                                     maskT_sb[:, m * 128:(m + 1) * 128])
                                pms.append(pm[:])
                            else:
                                pms.append(pexp[:, i * 128:(i + 1) * 128])
                        for i in range(nwin):
                            jb = qb - nwin + 1 + i
                            nc.tensor.matmul(acc[:, 0:128], V_sb[:, jb * 128:(jb + 1) * 128], pms[i],
                                             start=(i == 0), stop=(i == nwin - 1))
                        for i in range(nwin):
                            nc.tensor.matmul(acc[0:1, 128:256], onesj_sb[:], pms[i],
                                             start=(i == 0), stop=(i == nwin - 1))
                        rs = asb.tile([1, 128], dt.bfloat16, tag="rs")
                        with nc.allow_low_precision("softmax denominator reciprocal; 2e-2 rel-err budget"):
                            nc.vector.reciprocal(rs[:], acc[0:1, 128:256])
                        bsb = asb.tile([128, 128], dt.bfloat16, tag="bsb")
                        nc.gpsimd.partition_broadcast(bsb[:], rs[:])
                        nc.vector.tensor_mul(ynT[:, h * 128:(h + 1) * 128], acc[:, 0:128], bsb[:])
                    ostg = osb.tile([128, C], dt.float32, tag="ostg")
                    for cb4 in range(C // 512):
                        wps = wop.tile([128, 512], dt.float32, tag="wps")
                        for h in range(HPG):
                            nc.tensor.matmul(
                                wps[:], ynT[:, h * 128:(h + 1) * 128],
                                wo_sb[:, h * C + cb4 * 512: h * C + (cb4 + 1) * 512],
                                start=(h == 0), stop=(h == HPG - 1))
                        nc.any.tensor_copy(ostg[:, cb4 * 512:(cb4 + 1) * 512], wps[:])
                        if qb >= NQB - 2:
                            # drain the pipeline tail: chunked output DMA for the
                            # last two query blocks so the final transfer is short
                            nc.sync.dma_start(
                                out_d[qb * 128:(qb + 1) * 128, cb4 * 512:(cb4 + 1) * 512],
                                ostg[:, cb4 * 512:(cb4 + 1) * 512])
                    if qb < NQB - 2:
                        nc.sync.dma_start(out_d[qb * 128:(qb + 1) * 128, :], ostg[:])

    nc.compile()
    return nc


def _get_nc(t_len=T):
    if t_len not in _NC_CACHE:
        _NC_CACHE[t_len] = build_nc(t_len)
    return _NC_CACHE[t_len]


def host_inputs(x, Wq, Wk, Wv, Wo, t_len=T):
    """Per-core input shards (8 dicts)."""
    x = np.asarray(x, np.float32)
    Wq = np.asarray(Wq, np.float32)
    Wk = np.asarray(Wk, np.float32)
    Wv = np.asarray(Wv, np.float32)
    Wo = np.asarray(Wo, np.float32)
    cosT, sin_swap = _rope_tables(t_len)
    common = {
        "ident": np.eye(128, dtype=np.float32).astype(BF16),
        "cosq": (cosT * SCALE).astype(BF16),
        "sinq": (sin_swap * SCALE).astype(BF16),
        "cosk": cosT.astype(BF16),
        "sink": sin_swap.astype(BF16),
        "maskT": _band_maskT().astype(BF16),
        "ones_j": np.ones((128, 1), np.float32).astype(BF16),
    }
    in_maps = []
    for core in range(NCORES):
        b, hg = core // TPG, core % TPG
        m = dict(common)
        m["xT"] = np.ascontiguousarray(x[b, :t_len, :].T).astype(BF16)
        m["wq"] = np.ascontiguousarray(Wq[:, hg * HPG * HD:(hg + 1) * HPG * HD]).astype(BF16)
        m["wk"] = np.ascontiguousarray(Wk[:, hg * HD:(hg + 1) * HD]).astype(BF16)
        m["wv"] = np.ascontiguousarray(Wv[:, hg * HD:(hg + 1) * HD]).astype(BF16)
        m["wo"] = np.ascontiguousarray(Wo[hg * HPG * HD:(hg + 1) * HPG * HD, :]).astype(BF16)
        in_maps.append(m)
    return in_maps


def kernel(x, Wq, Wk, Wv, Wo):
    from concourse import bass_utils

    nc = _get_nc(T)
    in_maps = host_inputs(x, Wq, Wk, Wv, Wo, T)
    res = bass_utils.run_bass_kernel_spmd(nc, in_maps, core_ids=list(range(NCORES)))
    out = np.zeros((B, T, C), np.float32)
    for core in range(NCORES):
        out[core // TPG] += res.results[core]["out"]
    return out


def core_reference(x_b, Wq, Wk, Wv, Wo, hg, t_len=T):
    """Numpy reference of one core's partial output (f32 math, for dev tests)."""
    xb = np.asarray(x_b, np.float64)[:t_len]
    q = xb @ np.float64(Wq[:, hg * HPG * HD:(hg + 1) * HPG * HD])    # [T, 512]
    k = xb @ np.float64(Wk[:, hg * HD:(hg + 1) * HD])                # [T, 128]
    v = xb @ np.float64(Wv[:, hg * HD:(hg + 1) * HD])
    cosT, sin_swap = _rope_tables(t_len)
    cos = cosT.T.astype(np.float64)
    sinsw = sin_swap.T.astype(np.float64)

    def rope(z):
        zsw = np.concatenate([z[:, HD // 2:], z[:, :HD // 2]], axis=1)
        sgn = np.concatenate([sinsw[:, :HD // 2], sinsw[:, HD // 2:]], axis=1)
        return z * cos + zsw * sgn

    out = np.zeros((t_len, C), np.float64)
    i = np.arange(t_len)[:, None]
    j = np.arange(t_len)[None, :]
    allowed = (j <= i) & (i - j < WIN)
    kr = rope(k)
    for h in range(HPG):
        qh = rope(q[:, h * HD:(h + 1) * HD]) * SCALE
        s = qh @ kr.T
        s = np.where(allowed, s, -np.inf)
        p = np.exp(s - s.max(axis=1, keepdims=True))
        p /= p.sum(axis=1, keepdims=True)
        y = p @ v
        out += y @ np.float64(Wo[hg * HPG * HD + h * HD: hg * HPG * HD + (h + 1) * HD, :])
    return out.astype(np.float32)


# revision 45
# speedup vs baseline: 1.0586x; 1.0586x over previous
"""Trainium2 Bass kernel: causal sliding-window GQA self-attention.

Problem: B=2, T=2048, C=2048, 16 q-heads / 4 kv-heads, head_dim=128,
RoPE, sliding window 512, projections Wq/Wk/Wv/Wo.

Sharding: 8 cores = DP(batch=2) x TP(head-groups=4).  Core c handles
batch c//4 and q-heads [4*(c%4), 4*(c%4)+4) (one kv head c%4).  Each
core computes a partial output contribution [T, C]; the host sums the
4 head-group partials per batch.

Per-core kernel (all matmuls bf16, f32 accumulation):
  - host passes x[b]^T so the contraction dim (C) lands on partitions
  - Q^T/K^T [hd, t] via PE matmul, RoPE applied during PSUM eviction
    (scale 1/sqrt(hd) folded into the Q rope tables; the rotate-half is
    a cross-partition DVE multiply against sign-folded sin tables, and
    the final add runs on GpSimd to keep DVE off the critical path)
  - V^T computed, then PE-transposed to V [t, hd]
  - per (head, 128-query block): scores S^T [j, q] for the <=5 key
    blocks of the 640-wide causal window, one batched exp on ScalarE
    (no max subtraction -- max |score| measured 5.5 on this input
    distribution, exp stays tiny vs f32/bf16 range), 0/1 band-mask
    multiply (only the two edge key-blocks are not all-ones), PV and
    ones-row-sum matmuls as sequential accumulation groups in one PSUM
    bank (start=True clears has_written for the whole 2KB zero region,
    so groups sharing a bank must not interleave), bf16 reciprocal +
    GpSimd partition_broadcast for the 1/s normalization fused into the
    y^T eviction, then per-head Wo matmul accumulated over the 4 heads
    in PSUM.  DMA emission order is tuned so the shared 360GB/s DMA
    path stays ahead of the PE during the projection phase.

Timeline-sim (cost model) per-core exec: ~224.5us; rel err vs the f32
reference ~4.5e-3 (bf16 quantization of inputs/intermediates).
"""

import os
import sys

for _p in ("/opt/trn_rl_repo", "/root/.axon_site/_ro/trn_rl_repo"):
    if os.path.isdir(_p) and _p not in sys.path:
        sys.path.append(_p)

import numpy as np
import ml_dtypes

BF16 = ml_dtypes.bfloat16

B, T, C = 2, 2048, 2048
H, KVH, HD = 16, 4, 128
WIN = 512
ROPE_BASE = 10000.0
NCORES = 8
TPG = 4           # tensor-parallel group count (head groups)
HPG = H // TPG    # q-heads per core
SCALE = 1.0 / float(np.sqrt(np.float32(HD)))
NWINB = WIN // 128 + 1   # 5 key blocks cover the 640-wide window

_NC_CACHE = {}


def _rope_tables(t_len):
    # Match reference: angles computed in float32.
    inv = (1.0 / (np.float32(ROPE_BASE) ** (np.arange(0, HD, 2, dtype=np.float32) / np.float32(HD)))).astype(np.float32)
    ang = np.arange(t_len, dtype=np.float32)[None, :] * inv[:, None]   # [64, T]
    cosT = np.concatenate([np.cos(ang), np.cos(ang)], axis=0)          # [128, T]
    sinT = np.sin(ang)
    sin_swap = np.concatenate([-sinT, sinT], axis=0)                   # [128, T]
    return cosT.astype(np.float32), sin_swap.astype(np.float32)


def _band_maskT():
    # maskT[c, r] = 1 iff query row r may attend key col c of the
    # 640-wide window (c = j - (qs - 512)):  r+1 <= c <= r+512.
    r = np.arange(128)[None, :]
    c = np.arange(NWINB * 128)[:, None]
    return ((r + 1 <= c) & (c <= r + WIN)).astype(np.float32)          # [640, 128]


def build_nc(t_len=T):
    """Build + compile the per-core Bass module (SPMD, identical on all cores)."""
    import concourse.mybir as mybir
    import concourse.tile as tile
    from concourse import bacc
    from concourse import bass_isa

    dt = mybir.dt
    NQB = t_len // 128        # query/key blocks
    NCB = C // 128            # contraction blocks for projections
    NTB = t_len // 512        # 512-wide t-blocks for projections

    nc = bacc.Bacc("TRN2", target_bir_lowering=False, debug=False, num_devices=NCORES)

    def din(name, shape, d=dt.bfloat16):
        return nc.dram_tensor(name, shape, d, kind="ExternalInput").ap()

    xT_d = din("xT", [C, t_len])
    wq_d = din("wq", [C, HPG * HD])
    wk_d = din("wk", [C, HD])
    wv_d = din("wv", [C, HD])
    wo_d = din("wo", [HPG * HD, C])
    cosq_d = din("cosq", [HD, t_len])
    sinq_d = din("sinq", [HD, t_len])
    cosk_d = din("cosk", [HD, t_len])
    sink_d = din("sink", [HD, t_len])
    maskT_d = din("maskT", [NWINB * 128, 128])
    ident_d = din("ident", [128, 128])
    out_d = nc.dram_tensor("out", [t_len, C], dt.float32, kind="ExternalOutput").ap()

    with tile.TileContext(nc) as tc:
        with tc.tile_pool(name="persist", bufs=1) as pp:
            xT_sb = pp.tile([128, NCB * t_len], dt.bfloat16, tag="xT")
            wq_sb = pp.tile([128, NCB * HPG * HD], dt.bfloat16, tag="wq")
            wk_sb = pp.tile([128, NCB * HD], dt.bfloat16, tag="wk")
            wv_sb = pp.tile([128, NCB * HD], dt.bfloat16, tag="wv")
            wo_sb = pp.tile([128, HPG * C], dt.bfloat16, tag="wo")
            QT_sb = [pp.tile([128, t_len], dt.bfloat16, tag=f"QT{h}", name=f"QT{h}") for h in range(HPG)]
            KT_sb = pp.tile([128, t_len], dt.bfloat16, tag="KT")
            VT_sb = pp.tile([128, t_len], dt.bfloat16, tag="VT")
            V_sb = pp.tile([128, t_len], dt.bfloat16, tag="V")
            cosq_sb = pp.tile([128, t_len], dt.bfloat16, tag="cosq")
            sinq_sb = pp.tile([128, t_len], dt.bfloat16, tag="sinq")
            cosk_sb = pp.tile([128, t_len], dt.bfloat16, tag="cosk")
            sink_sb = pp.tile([128, t_len], dt.bfloat16, tag="sink")
            maskT_sb = pp.tile([128, NWINB * 128], dt.bfloat16, tag="maskT")
            ident_sb = pp.tile([128, 128], dt.bfloat16, tag="ident")

            # Load order matters: the shared DMA path is the projection-phase
            # rate limiter.  Small constants + rope tables + V/K weights first
            # (first consumers), then xT/wq interleaved, wo (attention-only) last.
            nc.sync.dma_start(ident_sb[:], ident_d[:])
            for cb in range(NCB):
                nc.sync.dma_start(xT_sb[:, cb * t_len:(cb + 1) * t_len], xT_d[cb * 128:(cb + 1) * 128, :])
                nc.sync.dma_start(wv_sb[:, cb * HD:(cb + 1) * HD], wv_d[cb * 128:(cb + 1) * 128, :])
                nc.sync.dma_start(wk_sb[:, cb * HD:(cb + 1) * HD], wk_d[cb * 128:(cb + 1) * 128, :])
            nc.sync.dma_start(cosk_sb[:], cosk_d[:])
            nc.sync.dma_start(sink_sb[:], sink_d[:])
            for cb in range(NCB):
                nc.sync.dma_start(wq_sb[:, cb * HPG * HD:(cb + 1) * HPG * HD], wq_d[cb * 128:(cb + 1) * 128, :])
                if cb == 3:
                    nc.sync.dma_start(cosq_sb[:], cosq_d[:])
                    nc.sync.dma_start(sinq_sb[:], sinq_d[:])
            for m in range(NWINB):
                nc.sync.dma_start(maskT_sb[:, m * 128:(m + 1) * 128], maskT_d[m * 128:(m + 1) * 128, :])
            for h in range(HPG):
                nc.sync.dma_start(wo_sb[:, h * C:(h + 1) * C], wo_d[h * 128:(h + 1) * 128, :])

            # ---------------- projections ----------------
            # Order: V -> V-transpose -> K -> Q(head-major).  Attention for
            # (h=0, qb=0) unblocks as soon as the first Q eviction lands.
            with tc.tile_pool(name="proj_ps", bufs=6, space="PSUM") as pps, \
                 tc.tile_pool(name="rope_scr", bufs=6) as rsc:

                def rope_evict(ps, dst, cos_sb, sin_sb, tb):
                    sl = slice(tb * 512, (tb + 1) * 512)
                    t1 = rsc.tile([128, 512], dt.float32, tag="t1")
                    t2 = rsc.tile([128, 512], dt.float32, tag="t2")
                    nc.vector.tensor_mul(t1[:], ps[:], cos_sb[:, sl])
                    nc.vector.tensor_mul(t2[0:64, :], ps[64:128, :], sin_sb[0:64, sl])
                    nc.vector.tensor_mul(t2[64:128, :], ps[0:64, :], sin_sb[64:128, sl])
                    nc.gpsimd.tensor_add(dst, t1[:], t2[:])

                for tb in range(NTB):
                    ps = pps.tile([128, 512], dt.float32, tag="ps", name="ps")
                    for cb in range(NCB):
                        nc.tensor.matmul(
                            ps[:], wv_sb[:, cb * HD:(cb + 1) * HD],
                            xT_sb[:, cb * t_len + tb * 512: cb * t_len + (tb + 1) * 512],
                            start=(cb == 0), stop=(cb == NCB - 1))
                    nc.any.tensor_copy(VT_sb[:, tb * 512:(tb + 1) * 512], ps[:])
                    ps = pps.tile([128, 512], dt.float32, tag="ps", name="ps")
                    for cb in range(NCB):
                        nc.tensor.matmul(
                            ps[:], wk_sb[:, cb * HD:(cb + 1) * HD],
                            xT_sb[:, cb * t_len + tb * 512: cb * t_len + (tb + 1) * 512],
                            start=(cb == 0), stop=(cb == NCB - 1))
                    rope_evict(ps, KT_sb[:, tb * 512:(tb + 1) * 512], cosk_sb, sink_sb, tb)
                with tc.tile_pool(name="tr_ps", bufs=2, space="PSUM") as tps:
                    for jb in range(NQB):
                        tp = tps.tile([128, 128], dt.bfloat16, tag="tp")
                        nc.tensor.transpose(tp[:], VT_sb[:, jb * 128:(jb + 1) * 128], ident_sb[:])
                        nc.any.tensor_copy(V_sb[:, jb * 128:(jb + 1) * 128], tp[:])
                for h in range(HPG):
                    for tb in range(NTB):
                        ps = pps.tile([128, 512], dt.float32, tag="ps")
                        for cb in range(NCB):
                            nc.tensor.matmul(
                                ps[:],
                                wq_sb[:, cb * HPG * HD + h * HD: cb * HPG * HD + (h + 1) * HD],
                                xT_sb[:, cb * t_len + tb * 512: cb * t_len + (tb + 1) * 512],
                                start=(cb == 0), stop=(cb == NCB - 1))
                        rope_evict(ps, QT_sb[h][:, tb * 512:(tb + 1) * 512],
                                   cosq_sb, sinq_sb, tb)

            # ---------------- attention + Wo ----------------
            with tc.tile_pool(name="st_ps", bufs=2, space="PSUM") as stp, \
                 tc.tile_pool(name="acc_ps", bufs=2, space="PSUM") as accp, \
                 tc.tile_pool(name="wo_ps", bufs=2, space="PSUM") as wop, \
                 tc.tile_pool(name="attn_sb", bufs=16) as asb, \
                 tc.tile_pool(name="yn_sb", bufs=2) as ysb, \
                 tc.tile_pool(name="out_sb", bufs=2) as osb:
                Exp = mybir.ActivationFunctionType.Exp
                for qb in range(NQB):
                    nwin = min(qb, NWINB - 1) + 1
                    ynT = ysb.tile([128, HPG * 128], dt.bfloat16, tag="ynT")
                    for h in range(HPG):
                        qsl = slice(qb * 128, (qb + 1) * 128)
                        st = stp.tile([128, NWINB * 128], dt.float32, tag="st")
                        for i in range(nwin):
                            jb = qb - nwin + 1 + i
                            nc.tensor.matmul(
                                st[:, i * 128:(i + 1) * 128],
                                KT_sb[:, jb * 128:(jb + 1) * 128],
                                QT_sb[h][:, qsl], start=True, stop=True)
                        acc = accp.tile([128, 128], dt.float32, tag="acc")
                        pexp = asb.tile([128, NWINB * 128], dt.bfloat16, tag="pexp")
                        nc.scalar.activation(pexp[:, 0:nwin * 128], st[:, 0:nwin * 128], Exp)
                        pms = []
                        for i in range(nwin):
                            m = i + NWINB - nwin
                            if m == 0 or m == NWINB - 1:
                                pm = asb.tile([128, 128], dt.bfloat16, tag="pmask")
                                nc.vector.tensor_mul(pm[:], pexp[:, i * 128:(i + 1) * 128],
                                                     maskT_sb[:, m * 128:(m + 1) * 128])
                                pms.append(pm[:])
                            else:
                                pms.append(pexp[:, i * 128:(i + 1) * 128])
                        for i in range(nwin):
                            jb = qb - nwin + 1 + i
                            nc.tensor.matmul(acc[:, 0:128], V_sb[:, jb * 128:(jb + 1) * 128], pms[i],
                                             start=(i == 0), stop=(i == nwin - 1))
                        # softmax denominators off the PE: pairwise-add the P^T
                        # tiles on DVE (bf16 4x), then one GpSimd
                        # partition_all_reduce gives s broadcast to every
                        # partition; reciprocal feeds the y^T eviction multiply.
                        work = list(pms)
                        while len(work) > 1:
                            nxt = []
                            for a, b in zip(work[0::2], work[1::2]):
                                t = asb.tile([128, 128], dt.bfloat16, tag="padd", name="padd")
                                nc.vector.tensor_add(t[:], a, b)
                                nxt.append(t[:])
                            if len(work) % 2:
                                nxt.append(work[-1])
                            work = nxt
                        sbc = asb.tile([128, 128], dt.float32, tag="sbc")
                        nc.gpsimd.partition_all_reduce(sbc[:], work[0], channels=128,
                                                       reduce_op=bass_isa.ReduceOp.add)
                        rbc = asb.tile([128, 128], dt.bfloat16, tag="rbc")
                        with nc.allow_low_precision("softmax denominator reciprocal; 2e-2 rel-err budget"):
                            nc.vector.reciprocal(rbc[:], sbc[:])
                        nc.vector.tensor_mul(ynT[:, h * 128:(h + 1) * 128], acc[:, 0:128], rbc[:])
                    ostg = osb.tile([128, C], dt.float32, tag="ostg")
                    for cb4 in range(C // 512):
                        wps = wop.tile([128, 512], dt.float32, tag="wps")
                        for h in range(HPG):
                            nc.tensor.matmul(
                                wps[:], ynT[:, h * 128:(h + 1) * 128],
                                wo_sb[:, h * C + cb4 * 512: h * C + (cb4 + 1) * 512],
                                start=(h == 0), stop=(h == HPG - 1))
                        nc.any.tensor_copy(ostg[:, cb4 * 512:(cb4 + 1) * 512], wps[:])
                        if qb >= NQB - 2:
                            # drain the pipeline tail: chunked output DMA for the
                            # last two query blocks so the final transfer is short
                            nc.sync.dma_start(
                                out_d[qb * 128:(qb + 1) * 128, cb4 * 512:(cb4 + 1) * 512],
                                ostg[:, cb4 * 512:(cb4 + 1) * 512])
                    if qb < NQB - 2:
                        nc.sync.dma_start(out_d[qb * 128:(qb + 1) * 128, :], ostg[:])

    nc.compile()
    return nc


def _get_nc(t_len=T):
    if t_len not in _NC_CACHE:
        _NC_CACHE[t_len] = build_nc(t_len)
    return _NC_CACHE[t_len]


def host_inputs(x, Wq, Wk, Wv, Wo, t_len=T):
    """Per-core input shards (8 dicts)."""
    x = np.asarray(x, np.float32)
    Wq = np.asarray(Wq, np.float32)
    Wk = np.asarray(Wk, np.float32)
    Wv = np.asarray(Wv, np.float32)
    Wo = np.asarray(Wo, np.float32)
    cosT, sin_swap = _rope_tables(t_len)
    common = {
        "ident": np.eye(128, dtype=np.float32).astype(BF16),
        "cosq": (cosT * SCALE).astype(BF16),
        "sinq": (sin_swap * SCALE).astype(BF16),
        "cosk": cosT.astype(BF16),
        "sink": sin_swap.astype(BF16),
        "maskT": _band_maskT().astype(BF16),
    }
    in_maps = []
    for core in range(NCORES):
        b, hg = core // TPG, core % TPG
        m = dict(common)
        m["xT"] = np.ascontiguousarray(x[b, :t_len, :].T).astype(BF16)
        m["wq"] = np.ascontiguousarray(Wq[:, hg * HPG * HD:(hg + 1) * HPG * HD]).astype(BF16)
        m["wk"] = np.ascontiguousarray(Wk[:, hg * HD:(hg + 1) * HD]).astype(BF16)
        m["wv"] = np.ascontiguousarray(Wv[:, hg * HD:(hg + 1) * HD]).astype(BF16)
        m["wo"] = np.ascontiguousarray(Wo[hg * HPG * HD:(hg + 1) * HPG * HD, :]).astype(BF16)
        in_maps.append(m)
    return in_maps


def kernel(x, Wq, Wk, Wv, Wo):
    from concourse import bass_utils

    nc = _get_nc(T)
    in_maps = host_inputs(x, Wq, Wk, Wv, Wo, T)
    res = bass_utils.run_bass_kernel_spmd(nc, in_maps, core_ids=list(range(NCORES)))
    out = np.zeros((B, T, C), np.float32)
    for core in range(NCORES):
        out[core // TPG] += res.results[core]["out"]
    return out


def core_reference(x_b, Wq, Wk, Wv, Wo, hg, t_len=T):
    """Numpy reference of one core's partial output (f32 math, for dev tests)."""
    xb = np.asarray(x_b, np.float64)[:t_len]
    q = xb @ np.float64(Wq[:, hg * HPG * HD:(hg + 1) * HPG * HD])    # [T, 512]
    k = xb @ np.float64(Wk[:, hg * HD:(hg + 1) * HD])                # [T, 128]
    v = xb @ np.float64(Wv[:, hg * HD:(hg + 1) * HD])
    cosT, sin_swap = _rope_tables(t_len)
    cos = cosT.T.astype(np.float64)
    sinsw = sin_swap.T.astype(np.float64)

    def rope(z):
        zsw = np.concatenate([z[:, HD // 2:], z[:, :HD // 2]], axis=1)
        sgn = np.concatenate([sinsw[:, :HD // 2], sinsw[:, HD // 2:]], axis=1)
        return z * cos + zsw * sgn

    out = np.zeros((t_len, C), np.float64)
    i = np.arange(t_len)[:, None]
    j = np.arange(t_len)[None, :]
    allowed = (j <= i) & (i - j < WIN)
    kr = rope(k)
    for h in range(HPG):
        qh = rope(q[:, h * HD:(h + 1) * HD]) * SCALE
        s = qh @ kr.T
        s = np.where(allowed, s, -np.inf)
        p = np.exp(s - s.max(axis=1, keepdims=True))
        p /= p.sum(axis=1, keepdims=True)
        y = p @ v
        out += y @ np.float64(Wo[hg * HPG * HD + h * HD: hg * HPG * HD + (h + 1) * HD, :])
    return out.astype(np.float32)


# revision 52
# speedup vs baseline: 1.0733x; 1.0139x over previous
"""Trainium2 Bass kernel: causal sliding-window GQA self-attention.

Problem: B=2, T=2048, C=2048, 16 q-heads / 4 kv-heads, head_dim=128,
RoPE, sliding window 512, projections Wq/Wk/Wv/Wo.

Sharding: 8 cores = DP(batch=2) x TP(head-groups=4).  Core c handles
batch c//4 and q-heads [4*(c%4), 4*(c%4)+4) (one kv head c%4).  Each
core computes a partial output contribution [T, C]; the host sums the
4 head-group partials per batch.

Per-core kernel (all matmuls bf16, f32 accumulation):
  - host passes x[b]^T so the contraction dim (C) lands on partitions
  - Q^T/K^T [hd, t] via PE matmul, RoPE applied during PSUM eviction
    (scale 1/sqrt(hd) folded into the Q rope tables; the rotate-half is
    a cross-partition DVE multiply against sign-folded sin tables, and
    the final add runs on GpSimd to keep DVE off the critical path)
  - V^T computed, then PE-transposed to V [t, hd]
  - per (head, 128-query block): scores S^T [j, q] for the <=5 key
    blocks of the 640-wide causal window, one batched exp on ScalarE
    (no max subtraction -- max |score| measured 5.5 on this input
    distribution, exp stays tiny vs f32/bf16 range), 0/1 band-mask
    multiply (only the two edge key-blocks are not all-ones), PV
    matmuls as one PSUM accumulation group.  Softmax denominators stay
    off the PE entirely: the P^T tiles are pairwise-added on DVE (bf16
    fast mode) and one GpSimd partition_all_reduce produces the
    per-query sums already broadcast across every partition; a bf16
    reciprocal then feeds the y^T eviction multiply (this sidesteps the
    partition-vs-free-dim mismatch that otherwise forces transposes).
    Per-head Wo matmuls accumulate over the 4 heads in PSUM.  DMA
    emission order is tuned so the shared 360GB/s DMA path stays ahead
    of the PE during the projection phase.

Timeline-sim (cost model) per-core exec: ~212.1us (PE busy ~170us, at
the warm bf16 roofline for this instruction mix); rel err vs the f32
reference ~4.5e-3 (bf16 quantization of inputs/intermediates).
"""

import os
import sys

for _p in ("/opt/trn_rl_repo", "/root/.axon_site/_ro/trn_rl_repo"):
    if os.path.isdir(_p) and _p not in sys.path:
        sys.path.append(_p)

import numpy as np
import ml_dtypes

BF16 = ml_dtypes.bfloat16

B, T, C = 2, 2048, 2048
H, KVH, HD = 16, 4, 128
WIN = 512
ROPE_BASE = 10000.0
NCORES = 8
TPG = 4           # tensor-parallel group count (head groups)
HPG = H // TPG    # q-heads per core
SCALE = 1.0 / float(np.sqrt(np.float32(HD)))
NWINB = WIN // 128 + 1   # 5 key blocks cover the 640-wide window

_NC_CACHE = {}


def _rope_tables(t_len):
    # Match reference: angles computed in float32.
    inv = (1.0 / (np.float32(ROPE_BASE) ** (np.arange(0, HD, 2, dtype=np.float32) / np.float32(HD)))).astype(np.float32)
    ang = np.arange(t_len, dtype=np.float32)[None, :] * inv[:, None]   # [64, T]
    cosT = np.concatenate([np.cos(ang), np.cos(ang)], axis=0)          # [128, T]
    sinT = np.sin(ang)
    sin_swap = np.concatenate([-sinT, sinT], axis=0)                   # [128, T]
    return cosT.astype(np.float32), sin_swap.astype(np.float32)


def _band_maskT():
    # maskT[c, r] = 1 iff query row r may attend key col c of the
    # 640-wide window (c = j - (qs - 512)):  r+1 <= c <= r+512.
    r = np.arange(128)[None, :]
    c = np.arange(NWINB * 128)[:, None]
    return ((r + 1 <= c) & (c <= r + WIN)).astype(np.float32)          # [640, 128]


def build_nc(t_len=T):
    """Build + compile the per-core Bass module (SPMD, identical on all cores)."""
    import concourse.mybir as mybir
    import concourse.tile as tile
    from concourse import bacc
    from concourse import bass_isa

    dt = mybir.dt
    NQB = t_len // 128        # query/key blocks
    NCB = C // 128            # contraction blocks for projections
    NTB = t_len // 512        # 512-wide t-blocks for projections

    nc = bacc.Bacc("TRN2", target_bir_lowering=False, debug=False, num_devices=NCORES)

    def din(name, shape, d=dt.bfloat16):
        return nc.dram_tensor(name, shape, d, kind="ExternalInput").ap()

    xT_d = din("xT", [C, t_len])
    wq_d = din("wq", [C, HPG * HD])
    wk_d = din("wk", [C, HD])
    wv_d = din("wv", [C, HD])
    wo_d = din("wo", [HPG * HD, C])
    cosq_d = din("cosq", [HD, t_len])
    sinq_d = din("sinq", [HD, t_len])
    cosk_d = din("cosk", [HD, t_len])
    sink_d = din("sink", [HD, t_len])
    maskT_d = din("maskT", [NWINB * 128, 128])
    ident_d = din("ident", [128, 128])
    out_d = nc.dram_tensor("out", [t_len, C], dt.float32, kind="ExternalOutput").ap()

    with tile.TileContext(nc) as tc:
        with tc.tile_pool(name="persist", bufs=1) as pp:
            xT_sb = pp.tile([128, NCB * t_len], dt.bfloat16, tag="xT")
            wq_sb = pp.tile([128, NCB * HPG * HD], dt.bfloat16, tag="wq")
            wk_sb = pp.tile([128, NCB * HD], dt.bfloat16, tag="wk")
            wv_sb = pp.tile([128, NCB * HD], dt.bfloat16, tag="wv")
            wo_sb = pp.tile([128, HPG * C], dt.bfloat16, tag="wo")
            QT_sb = [[pp.tile([128, 512], dt.bfloat16, tag=f"QT{h}_{tb}", name=f"QT{h}_{tb}")
                      for tb in range(NTB)] for h in range(HPG)]
            KT_sb = pp.tile([128, t_len], dt.bfloat16, tag="KT")
            VT_sb = pp.tile([128, t_len], dt.bfloat16, tag="VT")
            V_sb = pp.tile([128, t_len], dt.bfloat16, tag="V")
            cosq_sb = pp.tile([128, t_len], dt.bfloat16, tag="cosq")
            sinq_sb = pp.tile([128, t_len], dt.bfloat16, tag="sinq")
            cosk_sb = pp.tile([128, t_len], dt.bfloat16, tag="cosk")
            sink_sb = pp.tile([128, t_len], dt.bfloat16, tag="sink")
            maskT_sb = pp.tile([128, NWINB * 128], dt.bfloat16, tag="maskT")
            ident_sb = pp.tile([128, 128], dt.bfloat16, tag="ident")

            # Load order matters: the shared DMA path is the projection-phase
            # rate limiter.  Small constants + rope tables + V/K weights first
            # (first consumers), then xT/wq interleaved, wo (attention-only) last.
            nc.sync.dma_start(ident_sb[:], ident_d[:])
            for cb in range(NCB):
                nc.sync.dma_start(xT_sb[:, cb * t_len:(cb + 1) * t_len], xT_d[cb * 128:(cb + 1) * 128, :])
                nc.sync.dma_start(wv_sb[:, cb * HD:(cb + 1) * HD], wv_d[cb * 128:(cb + 1) * 128, :])
                nc.sync.dma_start(wk_sb[:, cb * HD:(cb + 1) * HD], wk_d[cb * 128:(cb + 1) * 128, :])
            nc.sync.dma_start(cosk_sb[:], cosk_d[:])
            nc.sync.dma_start(sink_sb[:], sink_d[:])
            for cb in range(NCB):
                nc.sync.dma_start(wq_sb[:, cb * HPG * HD:(cb + 1) * HPG * HD], wq_d[cb * 128:(cb + 1) * 128, :])
                if cb == 3:
                    nc.sync.dma_start(cosq_sb[:], cosq_d[:])
                    nc.sync.dma_start(sinq_sb[:], sinq_d[:])
            for m in range(NWINB):
                nc.sync.dma_start(maskT_sb[:, m * 128:(m + 1) * 128], maskT_d[m * 128:(m + 1) * 128, :])
            for h in range(HPG):
                nc.sync.dma_start(wo_sb[:, h * C:(h + 1) * C], wo_d[h * 128:(h + 1) * 128, :])

            # ---------------- projections ----------------
            with tc.tile_pool(name="proj_ps", bufs=7, space="PSUM") as pps, \
                 tc.tile_pool(name="rope_scr", bufs=4) as rsc:

                def rope_evict(ps, dst, cos_sb, sin_sb, tb):
                    sl = slice(tb * 512, (tb + 1) * 512)
                    t1 = rsc.tile([128, 512], dt.float32, tag="t1")
                    t2 = rsc.tile([128, 512], dt.float32, tag="t2")
                    nc.vector.tensor_mul(t1[:], ps[:], cos_sb[:, sl])
                    nc.vector.tensor_mul(t2[0:64, :], ps[64:128, :], sin_sb[0:64, sl])
                    nc.vector.tensor_mul(t2[64:128, :], ps[0:64, :], sin_sb[64:128, sl])
                    nc.gpsimd.tensor_add(dst, t1[:], t2[:])

                for tb in range(NTB):
                    ps = pps.tile([128, 512], dt.float32, tag="ps", name="ps")
                    for cb in range(NCB):
                        nc.tensor.matmul(
                            ps[:], wv_sb[:, cb * HD:(cb + 1) * HD],
                            xT_sb[:, cb * t_len + tb * 512: cb * t_len + (tb + 1) * 512],
                            start=(cb == 0), stop=(cb == NCB - 1))
                    nc.any.tensor_copy(VT_sb[:, tb * 512:(tb + 1) * 512], ps[:])
                    ps = pps.tile([128, 512], dt.float32, tag="ps", name="ps")
                    for cb in range(NCB):
                        nc.tensor.matmul(
                            ps[:], wk_sb[:, cb * HD:(cb + 1) * HD],
                            xT_sb[:, cb * t_len + tb * 512: cb * t_len + (tb + 1) * 512],
                            start=(cb == 0), stop=(cb == NCB - 1))
                    rope_evict(ps, KT_sb[:, tb * 512:(tb + 1) * 512], cosk_sb, sink_sb, tb)
                with tc.tile_pool(name="tr_ps", bufs=1, space="PSUM") as tps:
                    for jb in range(NQB):
                        tp = tps.tile([128, 128], dt.bfloat16, tag="tp")
                        nc.tensor.transpose(tp[:], VT_sb[:, jb * 128:(jb + 1) * 128], ident_sb[:])
                        nc.any.tensor_copy(V_sb[:, jb * 128:(jb + 1) * 128], tp[:])
                for tb in range(NTB):
                    for h in range(HPG):
                        ps = pps.tile([128, 512], dt.float32, tag="ps")
                        for cb in range(NCB):
                            nc.tensor.matmul(
                                ps[:],
                                wq_sb[:, cb * HPG * HD + h * HD: cb * HPG * HD + (h + 1) * HD],
                                xT_sb[:, cb * t_len + tb * 512: cb * t_len + (tb + 1) * 512],
                                start=(cb == 0), stop=(cb == NCB - 1))
                        rope_evict(ps, QT_sb[h][tb][:], cosq_sb, sinq_sb, tb)

            # ---------------- attention + Wo ----------------
            with tc.tile_pool(name="st_ps", bufs=2, space="PSUM") as stp, \
                 tc.tile_pool(name="acc_ps", bufs=2, space="PSUM") as accp, \
                 tc.tile_pool(name="wo_ps", bufs=2, space="PSUM") as wop, \
                 tc.tile_pool(name="attn_sb", bufs=16) as asb, \
                 tc.tile_pool(name="yn_sb", bufs=2) as ysb, \
                 tc.tile_pool(name="out_sb", bufs=2) as osb:
                Exp = mybir.ActivationFunctionType.Exp
                for qb in range(NQB):
                    nwin = min(qb, NWINB - 1) + 1
                    ynT = ysb.tile([128, HPG * 128], dt.bfloat16, tag="ynT")
                    for h in range(HPG):
                        qt = QT_sb[h][qb // 4]
                        qsl = slice((qb % 4) * 128, (qb % 4 + 1) * 128)
                        st = stp.tile([128, NWINB * 128], dt.float32, tag="st")
                        for i in range(nwin):
                            jb = qb - nwin + 1 + i
                            nc.tensor.matmul(
                                st[:, i * 128:(i + 1) * 128],
                                KT_sb[:, jb * 128:(jb + 1) * 128],
                                qt[:, qsl], start=True, stop=True)
                        acc = accp.tile([128, 128], dt.float32, tag="acc")
                        pexp = asb.tile([128, NWINB * 128], dt.bfloat16, tag="pexp")
                        nc.scalar.activation(pexp[:, 0:nwin * 128], st[:, 0:nwin * 128], Exp)
                        pms = []
                        for i in range(nwin):
                            m = i + NWINB - nwin
                            if m == 0 or m == NWINB - 1:
                                pm = asb.tile([128, 128], dt.bfloat16, tag="pmask")
                                nc.vector.tensor_mul(pm[:], pexp[:, i * 128:(i + 1) * 128],
                                                     maskT_sb[:, m * 128:(m + 1) * 128])
                                pms.append(pm[:])
                            else:
                                pms.append(pexp[:, i * 128:(i + 1) * 128])
                        for i in range(nwin):
                            jb = qb - nwin + 1 + i
                            nc.tensor.matmul(acc[:], V_sb[:, jb * 128:(jb + 1) * 128], pms[i],
                                             start=(i == 0), stop=(i == nwin - 1))
                        work = list(pms)
                        while len(work) > 1:
                            nxt = []
                            for a, b in zip(work[0::2], work[1::2]):
                                t = asb.tile([128, 128], dt.bfloat16, tag="padd", name="padd")
                                nc.vector.tensor_add(t[:], a, b)
                                nxt.append(t[:])
                            if len(work) % 2:
                                nxt.append(work[-1])
                            work = nxt
                        sbc = asb.tile([128, 128], dt.float32, tag="sbc")
                        nc.gpsimd.partition_all_reduce(sbc[:], work[0], channels=128,
                                                       reduce_op=bass_isa.ReduceOp.add)
                        rbc = asb.tile([128, 128], dt.bfloat16, tag="rbc")
                        with nc.allow_low_precision("softmax denominator reciprocal; 2e-2 rel-err budget"):
                            nc.vector.reciprocal(rbc[:], sbc[:])
                        nc.vector.tensor_mul(ynT[:, h * 128:(h + 1) * 128], acc[:], rbc[:])
                    ostg = osb.tile([128, C], dt.float32, tag="ostg")
                    for cb4 in range(C // 512):
                        wps = wop.tile([128, 512], dt.float32, tag="wps")
                        for hh in range(HPG):
                            nc.tensor.matmul(
                                wps[:], ynT[:, hh * 128:(hh + 1) * 128],
                                wo_sb[:, hh * C + cb4 * 512: hh * C + (cb4 + 1) * 512],
                                start=(hh == 0), stop=(hh == HPG - 1))
                        nc.any.tensor_copy(ostg[:, cb4 * 512:(cb4 + 1) * 512], wps[:])
                        if qb >= NQB - 2:
                            nc.sync.dma_start(
                                out_d[qb * 128:(qb + 1) * 128, cb4 * 512:(cb4 + 1) * 512],
                                ostg[:, cb4 * 512:(cb4 + 1) * 512])
                    if qb < NQB - 2:
                        nc.sync.dma_start(out_d[qb * 128:(qb + 1) * 128, :], ostg[:])

    nc.compile()
    return nc


def _get_nc(t_len=T):
    if t_len not in _NC_CACHE:
        _NC_CACHE[t_len] = build_nc(t_len)
    return _NC_CACHE[t_len]


def host_inputs(x, Wq, Wk, Wv, Wo, t_len=T):
    """Per-core input shards (8 dicts)."""
    x = np.asarray(x, np.float32)
    Wq = np.asarray(Wq, np.float32)
    Wk = np.asarray(Wk, np.float32)
    Wv = np.asarray(Wv, np.float32)
    Wo = np.asarray(Wo, np.float32)
    cosT, sin_swap = _rope_tables(t_len)
    common = {
        "ident": np.eye(128, dtype=np.float32).astype(BF16),
        "cosq": (cosT * SCALE).astype(BF16),
        "sinq": (sin_swap * SCALE).astype(BF16),
        "cosk": cosT.astype(BF16),
        "sink": sin_swap.astype(BF16),
        "maskT": _band_maskT().astype(BF16),
    }
    in_maps = []
    for core in range(NCORES):
        b, hg = core // TPG, core % TPG
        m = dict(common)
        m["xT"] = np.ascontiguousarray(x[b, :t_len, :].T).astype(BF16)
        m["wq"] = np.ascontiguousarray(Wq[:, hg * HPG * HD:(hg + 1) * HPG * HD]).astype(BF16)
        m["wk"] = np.ascontiguousarray(Wk[:, hg * HD:(hg + 1) * HD]).astype(BF16)
        m["wv"] = np.ascontiguousarray(Wv[:, hg * HD:(hg + 1) * HD]).astype(BF16)
        m["wo"] = np.ascontiguousarray(Wo[hg * HPG * HD:(hg + 1) * HPG * HD, :]).astype(BF16)
        in_maps.append(m)
    return in_maps


def kernel(x, Wq, Wk, Wv, Wo):
    from concourse import bass_utils

    nc = _get_nc(T)
    in_maps = host_inputs(x, Wq, Wk, Wv, Wo, T)
    res = bass_utils.run_bass_kernel_spmd(nc, in_maps, core_ids=list(range(NCORES)))
    out = np.zeros((B, T, C), np.float32)
    for core in range(NCORES):
        out[core // TPG] += res.results[core]["out"]
    return out


def core_reference(x_b, Wq, Wk, Wv, Wo, hg, t_len=T):
    """Numpy reference of one core's partial output (f32 math, for dev tests)."""
    xb = np.asarray(x_b, np.float64)[:t_len]
    q = xb @ np.float64(Wq[:, hg * HPG * HD:(hg + 1) * HPG * HD])    # [T, 512]
    k = xb @ np.float64(Wk[:, hg * HD:(hg + 1) * HD])                # [T, 128]
    v = xb @ np.float64(Wv[:, hg * HD:(hg + 1) * HD])
    cosT, sin_swap = _rope_tables(t_len)
    cos = cosT.T.astype(np.float64)
    sinsw = sin_swap.T.astype(np.float64)

    def rope(z):
        zsw = np.concatenate([z[:, HD // 2:], z[:, :HD // 2]], axis=1)
        sgn = np.concatenate([sinsw[:, :HD // 2], sinsw[:, HD // 2:]], axis=1)
        return z * cos + zsw * sgn

    out = np.zeros((t_len, C), np.float64)
    i = np.arange(t_len)[:, None]
    j = np.arange(t_len)[None, :]
    allowed = (j <= i) & (i - j < WIN)
    kr = rope(k)
    for h in range(HPG):
        qh = rope(q[:, h * HD:(h + 1) * HD]) * SCALE
        s = qh @ kr.T
        s = np.where(allowed, s, -np.inf)
        p = np.exp(s - s.max(axis=1, keepdims=True))
        p /= p.sum(axis=1, keepdims=True)
        y = p @ v
        out += y @ np.float64(Wo[hg * HPG * HD + h * HD: hg * HPG * HD + (h + 1) * HD, :])
    return out.astype(np.float32)
